# revision 33
# baseline (speedup 1.0000x reference)
"""Trainium2 Bass kernel for nn_Baseline_GNN (gnn_message_passing).

Data-parallel over batch across 8 NeuronCores. Per-core pipeline (fp16
activations, fp32 PSUM/stats).

  per layer l (3x):
    AGG:   vT = (maskT + eps*I)_s @ h_s per sample via PE (stationary are
           h_row half slices, moving operand is maskT). PSUM -> SBUF fp16
           copies alternate ACT/DVE.
    Z1:    z1T = W1.T-chunks @ vT (PE) into 2-bank PSUM row-pair tiles;
           one wide ACT copy (1024) + two DVE bn_stats (512) per tile.
    BN1:   cross-core AllReduce of (sum, sumsq) per feature -> scale/shift
           (rstd via ACT Ln/Exp; all ACT funcs live in one table set).
    ELU:   u = max(n, min(exp(n),1)-1), n = s*z+t: 1 ACT exp + 2 DVE
           tensor_scalar (4x f16 mode) + 1 DVE tensor_tensor max (2x).
    Z2/BN2/ELU -> w; BN3 stats: row-sums via 4x DVE ts-accum, sumsq via
    one ACT Square+accum per feature chunk; BN3/ELU -> h'.
    h'T -> h_row slots via PE identity-matmul transposes (DMA XBAR
           transposes were tried and are dispatch-bound: 625ns/instr on
           the serialized HWDGE device); copies alternate ACT/DVE.
  final: xm = row-sum over roi (DVE reduce; 1/200 folded into Wm1),
         AllGather xm, replicated tiny MLP with local BN stats, y (256,2).

insert_act_table_loads is patched to emit exactly one activation-table
load (natural_log_exp_and_others covers exp/ln/copy/relu/square) instead
of per-func switches.

b1/b2/bm1 are mathematically dropped (train-mode BN subtracts the mean, so
per-feature constant biases cancel exactly).
"""
import os
import numpy as np
import ml_dtypes

import concourse.bass as bass
import concourse.mybir as mybir
import concourse.tile as tile
import concourse.bacc as bacc
from concourse.bass_utils import run_bass_kernel_spmd

F32 = mybir.dt.float32
F16 = mybir.dt.float16
AF = mybir.ActivationFunctionType
A = mybir.AluOpType

B, ROI, T, L = 256, 200, 512, 3
NCORES = 8
S = B // NCORES            # samples per core
RPC = S * ROI              # rows per core (6400)
FC = T // 128              # feature chunks (4)
NPAIR = 7                  # 6x1024 + 1x256 row-pair blocks
PAIRS = [(i * 1024, min(1024, RPC - i * 1024)) for i in range(NPAIR)]
PADC = RPC + 256           # padded free dim for transpose source buffer
CW = 1600                  # ELU chunk width
CCH = RPC // CW            # 4 chunks
NGRP = RPC // 128          # 50 row-groups
HW_ = 8                    # hflat ring depth (groups)
MW_ = 16                   # mask ring depth (groups)
NG = float(B * ROI)        # global BN row count
NLOC = float(RPC)
BN_EPS = 1e-5

LAYER_REP = int(os.environ.get("K_LAYER_REP", "1"))
SKIP_AR = os.environ.get("K_SKIP_AR", "") == "1"
NO_ACTPATCH = os.environ.get("K_NO_ACTPATCH", "") == "1"
RDMA_AR = os.environ.get("K_RDMA_AR", "0") == "1"
DENSE_PAIR = os.environ.get("K_DENSE_PAIR", "0") == "1"
EVAC_POOL = os.environ.get("K_EVAC_POOL", "0") == "1"  # INVALID on HW: Pool cannot read PSUM
MINADD_POOL = int(os.environ.get("K_MINADD_POOL", "0"))  # every Nth on Pool; 0=off
TBATCH = os.environ.get("K_TBATCH", "1") == "1"
DPS_BUFS = int(os.environ.get("K_DPS_BUFS", "2"))

_ACT_SET = "natural_log_exp_and_others"


def _row_segments(s):
    """Row-group segments (g, p, n) covering rows [s*ROI, (s+1)*ROI)."""
    r0, r1 = s * ROI, (s + 1) * ROI
    segs = []
    r = r0
    while r < r1:
        g, p = divmod(r, 128)
        n = min(128 - p, r1 - r)
        segs.append((g, p, n))
        r += n
    return segs


def _patch_act_tables():
    """Replace per-func act-table loads with one entry load of _ACT_SET."""
    if NO_ACTPATCH:
        return
    if getattr(bacc.Bacc.insert_act_table_loads, "_single_set", False):
        return
    orig = bacc.Bacc.insert_act_table_loads
    from concourse.hw_specs import get_activation_tables

    def patched(self):
        tables = list(get_activation_tables(self.m.arch).items())
        names = [n for n, _ in tables]
        if _ACT_SET not in names:
            return orig(self)
        idx = names.index(_ACT_SET)
        allowed = tables[idx][1]
        funcs = {
            i.func
            for blk in self.main_func.blocks
            for i in blk.instructions
            if isinstance(i, mybir.InstActivation)
        }
        if not funcs:
            return
        if not funcs <= allowed:
            return orig(self)
        load = mybir.InstLoadActFuncSet(
            name=self.get_next_instruction_name(), ins=[], outs=[],
            act_func_set_id=idx)
        load.engine = mybir.EngineType.Activation
        self.register_instruction(load)
        self.main_func.blocks[0].instructions.insert(0, load)

    patched._single_set = True
    bacc.Bacc.insert_act_table_loads = patched


def build_nc():
    _patch_act_tables()
    nc = bacc.Bacc("TRN2", target_bir_lowering=False, debug=False,
                   num_devices=NCORES)

    xr = nc.dram_tensor("xr", [S, ROI, T], F16, kind="ExternalInput")
    mk = nc.dram_tensor("mk", [L, S, ROI, ROI], F16, kind="ExternalInput")
    w12 = nc.dram_tensor("w12", [L, 2, 128, FC, T], F16, kind="ExternalInput")
    bnp = nc.dram_tensor("bnp", [L, 6, 128, FC], F32, kind="ExternalInput")
    wm1 = nc.dram_tensor("wm1", [128, FC, 256], F16, kind="ExternalInput")
    wm2 = nc.dram_tensor("wm2", [128, 2, 2], F16, kind="ExternalInput")
    fbn = nc.dram_tensor("fbn", [128, 5], F32, kind="ExternalInput")
    idm = nc.dram_tensor("idm", [128, 128], F16, kind="ExternalInput")
    y = nc.dram_tensor("y", [B, 2], F32, kind="ExternalOutput")

    with tile.TileContext(nc) as tc:
        with (
            tc.tile_pool(name="big", bufs=1) as big,
            tc.tile_pool(name="wts", bufs=1) as wts,
            tc.tile_pool(name="mskp", bufs=3) as mskp,
            tc.tile_pool(name="esc", bufs=4) as esc,
            tc.tile_pool(name="stp", bufs=2) as stp,
            tc.tile_pool(name="stt", bufs=2) as stt,
            tc.tile_pool(name="dram", bufs=1, space="DRAM") as dram,
            tc.tile_pool(name="aps", bufs=2, space="PSUM") as aps,
            tc.tile_pool(name="dps", bufs=DPS_BUFS, space="PSUM") as dps,
        ):
            # --- persistent big activation buffers ---
            bufA = big.tile([128, FC, RPC], F16)          # vT / z2T
            bufB = big.tile([128, FC, RPC], F16)          # z1T / wT
            bufC = big.tile([128, FC, PADC], F16)         # uT / h'T (padded)
            hrow = big.tile([128, 6, 2, T], F16)          # slots x (a,b) halves
            idt = big.tile([128, 128], F16)
            nc.vector.memset(bufC[:, :, RPC:], 0.0)
            bnpt = big.tile([128, L, 6, FC], F32)
            fbnt = big.tile([128, 5], F32)
            wm1t = big.tile([128, FC, 256], F16)
            wm2t = big.tile([128, 2, 2], F16)

            def load_aux():
                # deferred so layer-0's x/mask loads go first in the SP FIFO;
                # idt is first consumed by agg(1), bnpt by bn_sync(0), the
                # rest by the head
                nc.sync.dma_start(idt[:], idm.ap())
                nc.sync.dma_start(bnpt[:], bnp.ap().rearrange("l k p c -> p l k c"))
                nc.sync.dma_start(fbnt[:], fbn.ap())
                nc.sync.dma_start(wm1t[:], wm1.ap())
                nc.sync.dma_start(wm2t[:], wm2.ap())

            if RDMA_AR:
                ar_rsem = nc.alloc_semaphore("ar_rsem")
                ar_lsem = nc.alloc_semaphore("ar_lsem")
                ar_psem = nc.alloc_semaphore("ar_psem")
                slots = big.tile([128, 2, 8, 2 * FC], F32)
                # all cores must have entered this invocation before the sem
                # clears: a peer still in invocation N must not see cleared
                # counters or receive N+1 packets early. Dummy AllGather is a
                # full-group rendezvous; Pool-queue FIFO orders the clears
                # after its completion.
                bgin = dram.tile([1, 1], mybir.dt.uint8, name="bgin")
                bgout = dram.tile([NCORES, 1], mybir.dt.uint8, name="bgout",
                                  addr_space="Shared")
                nc.gpsimd.collective_compute(
                    "AllGather", A.bypass, ins=[bgin[:].opt()],
                    outs=[bgout[:].opt()],
                    replica_groups=[list(range(NCORES))])
                nc.gpsimd.sem_clear(ar_rsem)
                nc.gpsimd.sem_clear(ar_psem)
                nc.gpsimd.sem_clear(ar_lsem)
                ar_round = [0]

            def load_weights(l):
                wt = wts.tile([128, 2, FC, T], F16, name=f"wt{l}", tag="wt")
                nc.sync.dma_start(wt[:], w12.ap()[l].rearrange("w p c t -> p w c t"))
                return wt

            def bn_sync(st6, nchunks, l, gk, bek, tag, pay=None):
                """Aggregate bn_stats chunks, AllReduce raw (sum, sumsq),
                return (s,t) (128,FC). pay: optional pre-filled payload tile
                [128, 2FC] = (sum, sumsq) -- skips the aggregation chain."""
                msq = stt.tile([128, FC], F32, name=f"msq{tag}", tag="msq")
                if pay is None:
                    ag = stt.tile([128, FC, 2], F32, name=f"ag{tag}", tag="ag")
                    for fo in range(FC):
                        nc.vector.bn_aggr(
                            ag[:, fo], st6[:, fo, :nchunks].rearrange("p c s -> p (c s)"))
                    mean = ag[:, :, 0:1].rearrange("p c o -> p (c o)")
                    var = ag[:, :, 1:2].rearrange("p c o -> p (c o)")
                    pay = stt.tile([128, 2 * FC], F32, name=f"pay{tag}", tag="pay")
                    nc.vector.tensor_tensor(msq[:], mean, mean, A.mult)
                    nc.vector.tensor_tensor(pay[:, FC:], msq[:], var, A.add)
                    nc.vector.tensor_scalar(pay[:, FC:], pay[:, FC:], NLOC, 0.0,
                                            A.mult, A.add)
                    nc.vector.tensor_scalar(pay[:, :FC], mean, NLOC, 0.0,
                                            A.mult, A.add)
                gp = stt.tile([128, 2 * FC], F32, name=f"gp{tag}", tag="gp")
                if RDMA_AR and not SKIP_AR:
                    # XOR-slot RDMA all-gather of payloads + local reduce:
                    # sender s writes dest d = s^k at slot k, so receiver r's
                    # slot k holds the payload from core r^k -- bijective.
                    rnd = ar_round[0]
                    ar_round[0] += 1
                    par = rnd % 2
                    with tc.tile_critical(name=f"ar{tag}"):
                        for d in range(NCORES):
                            rd = [None] * 8
                            rd[d] = (0, d)
                            nc.gpsimd.remote_dma_broadcast(
                                slots[:, par, d, :], pay[:],
                                ar_rsem, ar_lsem, rdests=rd,
                            ).then_inc(ar_psem, 1)
                        nc.gpsimd.wait_ge(ar_psem, 8 * (rnd + 1))
                        nc.gpsimd.trigger_dma(count=8)
                        nc.vector.wait_ge(ar_rsem, 16 * (rnd + 1))
                        nc.vector.tensor_reduce(
                            gp[:],
                            slots[:, par].rearrange("p s v -> p v s"),
                            mybir.AxisListType.X, A.add)
                else:
                    bin_ = dram.tile([128, 2 * FC], F32, name=f"bin{tag}")
                    # payload DMAs ride the ACT queue: the SP queue carries the
                    # bulk x/mask prefetches and head-of-line blocks small
                    # latency-critical transfers
                    nc.scalar.dma_start(bin_[:], pay[:])
                    if SKIP_AR:
                        nc.scalar.dma_start(gp[:], bin_[:])
                        nc.vector.tensor_scalar(gp[:], gp[:], float(NCORES),
                                                0.0, A.mult, A.add)
                    else:
                        # AllGather + local reduce instead of AllReduce: the
                        # reduce collective costs ~1.875x a same-size gather
                        # (ring reduce-scatter+gather vs gather) and the
                        # payload is tiny, so gather+DVE-reduce wins.
                        bout = dram.tile([NCORES, 128, 2 * FC], F32,
                                         name=f"bout{tag}", addr_space="Shared")
                        nc.gpsimd.collective_compute(
                            "AllGather", A.bypass, ins=[bin_[:].opt()],
                            outs=[bout[:].opt()],
                            replica_groups=[list(range(NCORES))])
                        slt = stt.tile([128, NCORES, 2 * FC], F32,
                                       name=f"slt{tag}", tag="slt")
                        # Pool-queue FIFO after the collective is the data
                        # arrival guarantee (same pattern as the head gather)
                        nc.gpsimd.dma_start(
                            slt[:], bout[:].rearrange("r p v -> p r v"))
                        nc.vector.tensor_reduce(
                            gp[:], slt[:].rearrange("p r v -> p v r"),
                            mybir.AxisListType.X, A.add)
                # gp holds global (sum, sumsq). var = sumsq/NG - (sum/NG)^2 =
                # (NG*sumsq - sum^2)/NG^2; the 1/NG^2 folds into the Ln scale.
                vg = stt.tile([128, FC], F32, name=f"vg{tag}", tag="vg")
                nc.vector.tensor_tensor(msq[:], gp[:, :FC], gp[:, :FC], A.mult)
                nc.vector.scalar_tensor_tensor(vg[:], gp[:, FC:], NG,
                                               msq[:], A.mult, A.subtract)
                nc.vector.tensor_scalar(vg[:], vg[:], 1.0 / (NG * NG), BN_EPS,
                                        A.mult, A.add)
                # rstd = exp(-0.5*ln(var+eps)); ln & exp live in one table set
                nc.scalar.activation(vg[:], vg[:], AF.Ln, bias=0.0, scale=1.0)
                nc.scalar.activation(vg[:], vg[:], AF.Exp, bias=0.0, scale=-0.5)
                st_s = stt.tile([128, FC], F32, name=f"s{tag}", tag="s")
                st_t = stt.tile([128, FC], F32, name=f"t{tag}", tag="t")
                nc.vector.tensor_tensor(st_s[:], vg[:], bnpt[:, l, gk], A.mult)
                # t = be - mean*s = be - (sum/NG)*s
                nc.vector.scalar_tensor_tensor(msq[:], gp[:, :FC], 1.0 / NG,
                                               st_s[:], A.mult, A.mult)
                nc.vector.tensor_tensor(st_t[:], bnpt[:, l, bek], msq[:],
                                        A.subtract)
                return st_s, st_t

            def apply_elu(zT, uT, st_s, st_t, tag, swc=None, post_chunk=None):
                """u = ELU(n) = max(s*z+t, min(exp(n),1)-1).
                swc: optional [128, FC, CCH] accum of row-sums of u (BN3).
                post_chunk(c): emit follow-on ops for chunk c interleaved into
                the engine streams (keeps dependent work off the tail)."""
                for c in range(CCH):
                    off = c * CW
                    es, rs = [], []
                    for fc in range(FC):
                        src = zT[:, fc, off:off + CW]
                        sA = st_s[:, fc:fc + 1]
                        tA = st_t[:, fc:fc + 1]
                        e = esc.tile([128, CW], F16, name=f"e{tag}_{c}_{fc}",
                                     tag="eb")
                        r = esc.tile([128, CW], F16, name=f"r{tag}_{c}_{fc}",
                                     tag="eb")
                        # exps and independent linear branches first: queuing
                        # the exp-dependent e2 ops early would head-of-line
                        # block the r's on the DVE FIFO
                        nc.scalar.activation(e[:], src, AF.Exp, bias=tA, scale=sA)
                        nc.vector.tensor_scalar(r[:], src, sA, tA, A.mult, A.add)
                        es.append(e)
                        rs.append(r)
                    for fc in range(FC):
                        e, r = es[fc], rs[fc]
                        on_pool = MINADD_POOL > 0 and fc % MINADD_POOL == 0
                        eng = nc.gpsimd if on_pool else nc.vector
                        eng.tensor_scalar(e[:], e[:], 1.0, -1.0,
                                          A.min, A.add)
                        dst = uT[:, fc, off:off + CW]
                        nc.vector.tensor_tensor(dst, r[:], e[:], A.max)
                        if swc is not None:
                            # garbage out into the now-dead r tile: same-engine
                            # in-order WAR, saves an esc allocation per tile
                            nc.vector.tensor_scalar(r[:], dst, 1.0, 0.0,
                                                    A.mult, A.add,
                                                    accum_out=swc[:, fc, c:c + 1])
                    if post_chunk is not None:
                        post_chunk(c)

            def dense(wt, wi, srcT, dstT, st6):
                """dstT = (W.T @ srcT); PSUM -> SBUF evac + DVE bn_stats.
                DENSE_PAIR: 2-bank row-pair tiles with one wide ACT evac
                (912ns fixed ACT overhead amortized over 1024 cols)."""
                if DENSE_PAIR:
                    NP = 7  # 6x1024 + 1x256
                    for rp in range(NP):
                        off = rp * 1024
                        n = min(1024, RPC - off)
                        nb = (n + 511) // 512
                        for fo in range(FC):
                            ps = dps.tile([128, 2, 512], F32,
                                          name=f"dps{wi}_{rp}_{fo}", tag="dpst")
                            for b in range(nb):
                                w = min(512, n - b * 512)
                                for fi in range(FC):
                                    nc.tensor.matmul(
                                        ps[:, b, :w],
                                        wt[:, wi, fi, fo * 128:(fo + 1) * 128],
                                        srcT[:, fi, off + b * 512:off + b * 512 + w],
                                        start=(fi == 0), stop=(fi == FC - 1),
                                        skip_group_check=(b == 1))
                            if nb == 2:
                                nc.scalar.activation(
                                    dstT[:, fo, off:off + n].rearrange(
                                        "p (a b) -> p a b", a=2),
                                    ps[:], AF.Copy)
                            else:
                                nc.scalar.activation(dstT[:, fo, off:off + n],
                                                     ps[:, 0, :n], AF.Copy)
                            for b in range(nb):
                                w = min(512, n - b * 512)
                                nc.vector.bn_stats(
                                    st6[:, fo, 2 * rp + b],
                                    dstT[:, fo, off + b * 512:off + b * 512 + w])
                else:
                    # fo-paired 2-bank PSUM tiles: 8 matmuls (3.4us PE) per
                    # tile vs one wide 2-bank ACT evac (1.67us) -- the ring
                    # never stalls and the wide evac amortizes ACT's ~900ns
                    # fixed per-op cost over 1024 cols.
                    NBLK = 13
                    for rb in range(NBLK):
                        off = rb * 512
                        n = min(512, RPC - off)
                        for fp in range(FC // 2):
                            ps = dps.tile([128, 2, 512], F32,
                                          name=f"dps{wi}_{rb}_{fp}", tag="dpst")
                            for half in range(2):
                                fo = fp * 2 + half
                                for fi in range(FC):
                                    nc.tensor.matmul(
                                        ps[:, half, :n],
                                        wt[:, wi, fi, fo * 128:(fo + 1) * 128],
                                        srcT[:, fi, off:off + n],
                                        start=(fi == 0), stop=(fi == FC - 1),
                                        skip_group_check=(half == 1))
                            nc.scalar.activation(
                                dstT[:, fp * 2:fp * 2 + 2, off:off + n],
                                ps[:, :, :n], AF.Copy)
                            for half in range(2):
                                fo = fp * 2 + half
                                nc.vector.bn_stats(st6[:, fo, rb],
                                                   dstT[:, fo, off:off + n])

            def agg(l, first):
                """vT (bufA) = (maskT + eps*I) @ h per sample."""
                mag = mbg = None
                for s in range(S):
                    slot = s % 6
                    mslot = s % 4
                    if s % 4 == 0:
                        mag = mskp.tile([128, 4, ROI], F16,
                                        name=f"ma{l}_{s}", tag="ma")
                        mbg = mskp.tile([128, 4, ROI], F16,
                                        name=f"mb{l}_{s}", tag="mb")
                        nc.sync.dma_start(
                            mag[:], mk.ap()[l, s:s + 4, 0:128, :].rearrange(
                                "s j i -> j s i"))
                        nc.sync.dma_start(
                            mbg[:72], mk.ap()[l, s:s + 4, 128:200, :].rearrange(
                                "s j i -> j s i"))
                    ma = mag[:, mslot]
                    mb = mbg[:, mslot]
                    if first:
                        nc.sync.dma_start(hrow[:, slot, 0, :], xr.ap()[s, 0:128, :])
                        nc.sync.dma_start(hrow[0:72, slot, 1, :], xr.ap()[s, 128:200, :])
                    elif TBATCH:
                        # h'T -> hrow halves via PE identity transpose; all 8
                        # [128,128] blocks of a sample into ONE f16 psum bank,
                        # then a single strided evac (amortizes fixed op cost)
                        c0 = s * ROI
                        tp = dps.tile([128, 1024], F16,
                                      name=f"tp{l}_{s}", tag="dpst")
                        for fcx in range(FC):
                            nc.tensor.matmul(tp[:, fcx * 256:fcx * 256 + 128],
                                             bufC[:, fcx, c0:c0 + 128], idt[:],
                                             is_transpose=True,
                                             start=(fcx == 0), stop=False,
                                             skip_group_check=(fcx > 0))
                            nc.tensor.matmul(tp[:, fcx * 256 + 128:fcx * 256 + 256],
                                             bufC[:, fcx, c0 + 128:c0 + 256],
                                             idt[:], is_transpose=True,
                                             start=False, stop=(fcx == FC - 1),
                                             skip_group_check=True)
                        src3 = tp[:].rearrange("p (c h f) -> p h c f", c=FC, h=2)
                        dst = hrow[:, slot, :, :].rearrange(
                            "p h (c f) -> p h c f", c=FC)
                        if s % 2 == 0:
                            nc.scalar.activation(dst, src3, AF.Copy)
                        else:
                            nc.vector.tensor_scalar(dst, src3, 1.0, 0.0,
                                                    A.mult, A.add)
                    else:
                        for fcx in range(FC):
                            c0 = s * ROI
                            tp = dps.tile([128, 512], F16,
                                          name=f"tp{l}_{s}_{fcx}", tag="dpst")
                            nc.tensor.matmul(tp[:, 0:128],
                                             bufC[:, fcx, c0:c0 + 128], idt[:],
                                             is_transpose=True, start=True,
                                             stop=False)
                            nc.tensor.matmul(tp[:, 128:256],
                                             bufC[:, fcx, c0 + 128:c0 + 256],
                                             idt[:], is_transpose=True,
                                             start=False, stop=True,
                                             skip_group_check=True)
                            dst = hrow[:, slot, :, fcx * 128:(fcx + 1) * 128]
                            src3 = tp[:, 0:256].rearrange(
                                "p (h f) -> p h f", h=2)
                            if (s + fcx) % 2 == 0:
                                nc.scalar.activation(dst, src3, AF.Copy)
                            else:
                                nc.vector.tensor_scalar(dst, src3, 1.0, 0.0,
                                                        A.mult, A.add)
                    for half in range(2):
                        ps = aps.tile([128, 2, 512], F32,
                                      name=f"ap{l}_{s}_{half}", tag="apst")
                        for sub in range(2):
                            fcx = half * 2 + sub
                            nc.tensor.matmul(
                                ps[:, sub, :ROI],
                                hrow[:, slot, 0, fcx * 128:(fcx + 1) * 128],
                                ma, start=True, stop=False)
                            nc.tensor.matmul(
                                ps[:, sub, :ROI],
                                hrow[0:72, slot, 1, fcx * 128:(fcx + 1) * 128],
                                mb[0:72], start=False, stop=True,
                                skip_group_check=True)
                        # NOTE: Pool/GPSIMD cannot access PSUM on real HW
                        # (neuronxcc BIR verification rejects it) -- evacs can
                        # only ride ACT or DVE.
                        if s % 2 == 0:
                            nc.scalar.activation(
                                bufA[:, half * 2:half * 2 + 2,
                                     s * ROI:(s + 1) * ROI],
                                ps[:, :, :ROI], AF.Copy)
                        else:
                            nc.vector.tensor_scalar(
                                bufA[:, half * 2:half * 2 + 2,
                                     s * ROI:(s + 1) * ROI],
                                ps[:, :, :ROI], 1.0, 0.0, A.mult, A.add)

            # ================== main ==================
            xmT = big.tile([128, FC, S], F16)
            SPC = CW // ROI            # samples per ELU chunk (8)

            def head_reduce_chunk(c):
                # row-mean over roi for this ELU3 chunk, interleaved into the
                # DVE stream so the head's reduce isn't an exposed tail
                for fcx in range(FC):
                    with nc.allow_low_precision(reason="xm f16: head BN renormalizes"):
                        nc.vector.tensor_reduce(
                            xmT[:, fcx, c * SPC:(c + 1) * SPC],
                            bufC[:, fcx, c * CW:(c + 1) * CW].rearrange(
                                "p (s r) -> p s r", r=ROI),
                            mybir.AxisListType.X, A.add)

            first_iter = [True]
            lits = [ll % L for ll in range(L * LAYER_REP)]
            for li, l in enumerate(lits):
                wt = load_weights(l)
                agg(l, first=first_iter[0])
                if first_iter[0]:
                    load_aux()
                    first_iter[0] = False
                st6a = stp.tile([128, FC, 13, 6], F32, name="st6a", tag="st6")
                dense(wt, 0, bufA, bufB, st6a)
                s1, t1 = bn_sync(st6a, 13, l, 0, 1, f"a{l}")
                apply_elu(bufB, bufC, s1, t1, f"a{l}")
                st6b = stp.tile([128, FC, 13, 6], F32, name="st6b", tag="st6")
                dense(wt, 1, bufC, bufA, st6b)
                s2, t2 = bn_sync(st6b, 13, l, 2, 3, f"b{l}")
                swc = stt.tile([128, FC, CCH], F32, name=f"swc{l}", tag="swc")
                sqc = stt.tile([128, FC, CCH], F32, name=f"sqc{l}", tag="sqc")
                apply_elu(bufA, bufB, s2, t2, f"b{l}", swc=swc)
                # BN3 sumsq: ACT Square+accum for chunks 0..2; the last
                # chunk runs on the otherwise-idle DVE (tt square into f16
                # scratch + 4x ts-accum) so the stats reduce is not gated on
                # the ACT Square tail. (tensor_tensor_reduce would do it in
                # one op but crashes the HW worker -- do not reintroduce.)
                for c in range(CCH):
                    off = c * CW
                    for fo in range(FC):
                        if c < CCH - 1:
                            nc.scalar.activation(
                                bufA[:, fo, off:off + CW],
                                bufB[:, fo, off:off + CW], AF.Square,
                                accum_out=sqc[:, fo, c:c + 1])
                        else:
                            sq = esc.tile([128, CW], F16,
                                          name=f"sq{l}_{fo}", tag="eb")
                            nc.vector.tensor_tensor(
                                sq[:], bufB[:, fo, off:off + CW],
                                bufB[:, fo, off:off + CW], A.mult)
                            nc.vector.tensor_scalar(
                                sq[:], sq[:], 1.0, 0.0, A.mult, A.add,
                                accum_out=sqc[:, fo, c:c + 1])
                pay3 = stt.tile([128, 2 * FC], F32, name=f"pay3_{li}", tag="pay")
                nc.vector.tensor_reduce(pay3[:, :FC], swc[:],
                                        mybir.AxisListType.X, A.add)
                nc.vector.tensor_reduce(pay3[:, FC:], sqc[:],
                                        mybir.AxisListType.X, A.add)
                last = (li == len(lits) - 1)
                s3, t3 = bn_sync(None, 13, l, 4, 5, f"c{li}", pay=pay3)
                apply_elu(bufB, bufC, s3, t3, f"c{li}",
                          post_chunk=head_reduce_chunk if last else None)

            # ---- final head ----
            gin = dram.tile([128, FC * S], F16, name="gin")
            gout = dram.tile([NCORES, 128, FC * S], F16, name="gout",
                             addr_space="Shared")
            nc.scalar.dma_start(gin[:], xmT[:].rearrange("p c s -> p (c s)"))
            nc.gpsimd.collective_compute(
                "AllGather", A.bypass, ins=[gin[:].opt()], outs=[gout[:].opt()],
                replica_groups=[list(range(NCORES))])
            xa = big.tile([128, NCORES, FC, S], F16)
            # Pool-queue FIFO after the collective is the only data-arrival
            # guarantee -- the xa load MUST stay on gpsimd (a marker-fence +
            # HWDGE bulk load raced the AllGather data intermittently).
            # Plain r->partition-major pattern (cheap descriptors); the
            # fi-slice reordering is done by the matmul's strided moving AP.
            nc.gpsimd.dma_start(xa[:], gout[:].rearrange("r p f -> p r f"))
            # zm.T = Wm1.T @ xa  (fo=256 -> 2 chunks)
            zt = big.tile([128, 2, B], F32)
            st6f = stp.tile([128, 2, 1, 6], F32, name="st6f", tag="st6f")
            for fo in range(2):
                ps = aps.tile([128, 2, 512], F32, name=f"fps{fo}", tag="apst")
                for fi in range(FC):
                    nc.tensor.matmul(ps[:, 0, :B], wm1t[:, fi, fo * 128:(fo + 1) * 128],
                                     xa[:, :, fi, :], start=(fi == 0),
                                     stop=(fi == FC - 1))
                nc.scalar.activation(zt[:, fo, :], ps[:, 0, :B], AF.Copy)
                nc.vector.bn_stats(st6f[:, fo, 0], zt[:, fo, :])
            # local BN (all 256 rows present) + relu
            agf = stt.tile([128, 2, 2], F32, name="agf")
            for fo in range(2):
                nc.vector.bn_aggr(agf[:, fo], st6f[:, fo, 0])
            vgf = stt.tile([128, 2], F32, name="vgf")
            nc.vector.tensor_copy(vgf[:], agf[:, :, 1:2].rearrange("p c o -> p (c o)"))
            nc.vector.tensor_scalar(vgf[:], vgf[:], 1.0, BN_EPS, A.mult, A.add)
            nc.scalar.activation(vgf[:], vgf[:], AF.Ln, bias=0.0, scale=1.0)
            nc.scalar.activation(vgf[:], vgf[:], AF.Exp, bias=0.0, scale=-0.5)
            sf = stt.tile([128, 2], F32, name="sf")
            tf = stt.tile([128, 2], F32, name="tf")
            nc.vector.tensor_tensor(sf[:], vgf[:], fbnt[:, 0:2], A.mult)
            nc.vector.tensor_tensor(tf[:], agf[:, :, 0:1].rearrange("p c o -> p (c o)"), sf[:], A.mult)
            nc.vector.tensor_tensor(tf[:], fbnt[:, 2:4], tf[:], A.subtract)
            rt = big.tile([128, 2, B], F16)
            for fo in range(2):
                nc.scalar.activation(rt[:, fo, :], zt[:, fo, :], AF.Relu,
                                     bias=tf[:, fo:fo + 1], scale=sf[:, fo:fo + 1])
            psy = aps.tile([128, 2, 512], F32, name="psy", tag="apst")
            for fo in range(2):
                nc.tensor.matmul(psy[0:2, 0, :B], wm2t[:, fo, :], rt[:, fo, :],
                                 start=(fo == 0), stop=(fo == 1))
            ysb = big.tile([128, B], F32)
            nc.vector.tensor_scalar(ysb[0:2, :], psy[0:2, 0, :B], 1.0,
                                    fbnt[0:2, 4:5], A.mult, A.add)
            nc.sync.dma_start(y.ap().rearrange("b t -> t b"), ysb[0:2, :])
    nc.compile()
    return nc


_NC_CACHE = None


def _get_nc():
    global _NC_CACHE
    if _NC_CACHE is None:
        _NC_CACHE = build_nc()
    return _NC_CACHE


def _prep_inputs(x, a, eps, W1, W2, gl_, bl_, g1, be1, g2, be2,
                 gm, betam, Wm1, bm2, Wm2):
    f16 = np.float16
    mask = (np.asarray(a) != 0).astype(np.float32)          # [b, i, j]
    maskT = np.ascontiguousarray(mask.transpose(0, 2, 1))   # [b, j, i]
    eye = np.eye(ROI, dtype=np.float32)
    mk = np.empty((L, B, ROI, ROI), dtype=f16)
    for l in range(L):
        mk[l] = (maskT + float(eps[l]) * eye).astype(f16)
    x_row = np.asarray(x).astype(f16)                        # [b, roi, T]
    w12 = np.empty((L, 2, 128, FC, T), dtype=f16)
    for l in range(L):
        w12[l, 0] = np.asarray(W1[l]).reshape(FC, 128, T).transpose(1, 0, 2)
        w12[l, 1] = np.asarray(W2[l]).reshape(FC, 128, T).transpose(1, 0, 2)
    bnp = np.empty((L, 6, 128, FC), dtype=np.float32)
    for l in range(L):
        for k, p in enumerate((g1[l], be1[l], g2[l], be2[l], gl_[l], bl_[l])):
            bnp[l, k] = np.asarray(p).reshape(FC, 128).T
    wm1p = (np.asarray(Wm1) / ROI).reshape(FC, 128, 256).transpose(1, 0, 2).astype(f16)
    wm2p = np.asarray(Wm2).reshape(2, 128, 2).transpose(1, 0, 2).astype(f16)
    fbn = np.zeros((128, 5), dtype=np.float32)
    fbn[:, 0:2] = np.asarray(gm).reshape(2, 128).T
    fbn[:, 2:4] = np.asarray(betam).reshape(2, 128).T
    fbn[0:2, 4] = np.asarray(bm2)
    return x_row, mk, w12, bnp, wm1p, wm2p, fbn


def make_in_maps(inputs):
    x_row, mk, w12, bnp, wm1p, wm2p, fbn = _prep_inputs(
        inputs['x'], inputs['a'], inputs['eps'], inputs['W1'], inputs['W2'],
        inputs['gl'], inputs['bl'], inputs['g1'], inputs['be1'], inputs['g2'],
        inputs['be2'], inputs['gm'], inputs['betam'], inputs['Wm1'],
        inputs['bm2'], inputs['Wm2'])
    idm = np.eye(128, dtype=np.float16)
    in_maps = []
    for c in range(NCORES):
        sl = slice(c * S, (c + 1) * S)
        in_maps.append({
            "xr": np.ascontiguousarray(x_row[sl]),
            "mk": np.ascontiguousarray(mk[:, sl]),
            "w12": w12, "bnp": bnp, "wm1": wm1p, "wm2": wm2p, "fbn": fbn,
            "idm": idm,
        })
    return in_maps


def kernel(x, a, eps, W1, b1, g1, be1, W2, b2, g2, be2, gl, bl,
           Wm1, bm1, gm, betam, Wm2, bm2):
    in_maps = make_in_maps(dict(x=x, a=a, eps=eps, W1=W1, W2=W2, gl=gl, bl=bl,
                                g1=g1, be1=be1, g2=g2, be2=be2, gm=gm,
                                betam=betam, Wm1=Wm1, bm2=bm2, Wm2=Wm2))
    nc = _get_nc()
    res = run_bass_kernel_spmd(nc, in_maps, core_ids=list(range(NCORES)))
    return res.results[0]["y"].astype(np.float32)

